# revision 4
# baseline (speedup 1.0000x reference)
"""PINN (IRK tanh-MLP + 2nd-order forward AD) Trainium2 kernel.

Data-parallel over 8 NeuronCores: x is sharded along the collocation axis,
weights/IRK matrices replicated.  Per core the MLP [1,20,50,200,500,200,100]
is evaluated feature-major with three forward-mode streams (value h in
float32r, first/second input-derivatives dh/d2h in bf16), followed by the
output transform in batch-major layout and the IRK matmuls.
"""

import os
import sys

sys.path.insert(0, "/opt/trn_rl_repo")

import numpy as np
import ml_dtypes

import concourse.bass as bass
import concourse.mybir as mybir
import concourse.tile as tile
from concourse import bacc
from concourse.masks import make_identity

F32 = mybir.dt.float32
F32R = mybir.dt.float32r
BF16 = mybir.dt.bfloat16
AF = mybir.ActivationFunctionType
ALU = mybir.AluOpType

N_CORES = 8
N_TOTAL = 65536
NC = N_TOTAL // N_CORES  # 8192 samples per core
B = 512                  # batch tile (free dim per matmul)
T = NC // B              # 16 batch tiles per core
XC = NC // 128           # 64 x-columns per core
Q = 100
DT = 0.8
LAYERS = [1, 20, 50, 200, 500, 200, 100]


def _chunks(n):
    """Split a feature width into partition chunks of <=128: [(start, size)]."""
    out = []
    s = 0
    while s < n:
        sz = min(128, n - s)
        out.append((s, sz))
        s += sz
    return out


def build_kernel():
    nc = bacc.Bacc("TRN2", target_bir_lowering=False, debug=False,
                   num_devices=N_CORES)

    # ---- DRAM parameters -------------------------------------------------
    xrow_e = nc.declare_dram_parameter("xrow", [T, B], F32, isOutput=False)
    xcol_e = nc.declare_dram_parameter("xcol", [128, XC], F32, isOutput=False)
    wtr_e, wtb_e, bc_e = {}, {}, {}
    for l in range(1, 6):
        fi, fo = LAYERS[l], LAYERS[l + 1]
        kc = len(_chunks(fi))
        mc = len(_chunks(fo))
        wtr_e[l] = nc.declare_dram_parameter(f"wtr{l}", [128, kc * fo], F32,
                                             isOutput=False)
        wtb_e[l] = nc.declare_dram_parameter(f"wtb{l}", [128, kc * fo], BF16,
                                             isOutput=False)
        bc_e[l] = nc.declare_dram_parameter(f"bc{l}", [128, mc], F32,
                                            isOutput=False)
    # layer 0 specials: W0 as a row (lhsT) and as per-partition columns
    w0r_e = nc.declare_dram_parameter("w0r", [1, 20], F32, isOutput=False)
    w0cb_e = nc.declare_dram_parameter("w0cb", [128, 1], F32, isOutput=False)
    w0cn_e = nc.declare_dram_parameter("w0cn", [128, 1], F32, isOutput=False)
    b0c_e = nc.declare_dram_parameter("b0c", [128, 1], F32, isOutput=False)
    g12_e = nc.declare_dram_parameter("g12", [128, 2 * Q], F32, isOutput=False)
    u0_e = nc.declare_dram_parameter("U0", [NC, Q], F32, isOutput=True)
    u1_e = nc.declare_dram_parameter("U1", [NC, Q], F32, isOutput=True)

    from contextlib import ExitStack
    with tile.TileContext(nc) as tc, ExitStack() as es:
        wpool = es.enter_context(tc.tile_pool(name="weights", bufs=1))
        apool = es.enter_context(tc.tile_pool(name="acts", bufs=2))
        tpool = es.enter_context(tc.tile_pool(name="tmp", bufs=3))
        pmm = es.enter_context(tc.tile_pool(name="pmm", bufs=2, space="PSUM"))
        pmisc = es.enter_context(tc.tile_pool(name="pmisc", bufs=1,
                                              space="PSUM"))

        # ---- resident weights -------------------------------------------
        wtr, wtb, bc = {}, {}, {}
        for l in range(1, 6):
            fi, fo = LAYERS[l], LAYERS[l + 1]
            kc = len(_chunks(fi))
            mc = len(_chunks(fo))
            wtr[l] = wpool.tile([128, kc * fo], F32R, name=f"wtr{l}_sb")
            nc.gpsimd.dma_start(out=wtr[l][:, :], in_=wtr_e[l][:, :].bitcast(F32R))
            wtb[l] = wpool.tile([128, kc * fo], BF16, name=f"wtb{l}_sb")
            nc.gpsimd.dma_start(out=wtb[l][:, :], in_=wtb_e[l][:, :])
            bc[l] = wpool.tile([128, mc], F32, name=f"bc{l}_sb")
            nc.gpsimd.dma_start(out=bc[l][:, :], in_=bc_e[l][:, :])
        w0r = wpool.tile([1, 20], F32R, name="w0r_sb")
        nc.gpsimd.dma_start(out=w0r[:, :], in_=w0r_e[:, :].bitcast(F32R))
        w0cb = wpool.tile([128, 1], F32, name="w0cb_sb")
        nc.gpsimd.dma_start(out=w0cb[:, :], in_=w0cb_e[:, :])
        w0cn = wpool.tile([128, 1], F32, name="w0cn_sb")
        nc.gpsimd.dma_start(out=w0cn[:, :], in_=w0cn_e[:, :])
        b0c = wpool.tile([128, 1], F32, name="b0c_sb")
        nc.gpsimd.dma_start(out=b0c[:, :], in_=b0c_e[:, :])
        g12 = wpool.tile([128, 2 * Q], F32R, name="g12_sb")
        nc.gpsimd.dma_start(out=g12[:, :], in_=g12_e[:, :].bitcast(F32R))

        ident = wpool.tile([128, 128], F32, name="ident")
        make_identity(nc, ident[:, :])

        # per-partition x scalars for the whole core
        xcol = wpool.tile([128, XC], F32, name="xcol_sb")
        nc.gpsimd.dma_start(out=xcol[:, :], in_=xcol_e[:, :])
        xsq1 = wpool.tile([128, XC], F32, name="xsq1")
        nc.scalar.activation(xsq1[:, :], xcol[:, :], AF.Square)
        nc.vector.tensor_scalar_add(xsq1[:, :], xsq1[:, :], -1.0)
        x4 = wpool.tile([128, XC], F32, name="x4")
        nc.vector.tensor_scalar_mul(x4[:, :], xcol[:, :], 4.0)

        # ---- main loop over batch tiles ---------------------------------
        for t in range(T):
            xr = tpool.tile([1, B], F32R, name="xr", tag="xr")
            nc.gpsimd.dma_start(out=xr[:, :],
                                in_=xrow_e[t:t + 1, :].bitcast(F32R))

            # ---------------- layer 0 (1 -> 20) --------------------------
            w0 = LAYERS[1]  # 20
            ph0 = pmm.tile([128, B], F32, name="ph0", tag="ph")
            nc.tensor.matmul(ph0[0:w0, :], w0r[0:1, :], xr[0:1, :],
                             start=True, stop=True)
            h = apool.tile([128, B], F32R, name="h0", tag="h0")
            nc.scalar.activation(h[0:w0, :], ph0[0:w0, :], AF.Tanh,
                                 bias=b0c[0:w0, :])
            tbf = tpool.tile([128, B], BF16, name="tbf0", tag="tbf")
            nc.gpsimd.tensor_copy(tbf[0:w0, :], h[0:w0, :].bitcast(F32))
            ttb = tpool.tile([128, B], BF16, name="ttb0", tag="ttb")
            nc.scalar.activation(ttb[0:w0, :], h[0:w0, :].bitcast(F32),
                                 AF.Square)
            sbf = tpool.tile([128, B], BF16, name="sbf0", tag="sbf")
            nc.gpsimd.tensor_scalar(sbf[0:w0, :], ttb[0:w0, :], -1.0, 1.0,
                                    ALU.mult, ALU.add)
            dh = apool.tile([128, B], BF16, name="dh0", tag="dh0")
            nc.vector.tensor_scalar_mul(dh[0:w0, :], sbf[0:w0, :],
                                        w0cb[0:w0, :])
            r0 = tpool.tile([128, B], BF16, name="r0", tag="rbf")
            nc.vector.tensor_mul(r0[0:w0, :], tbf[0:w0, :], dh[0:w0, :])
            d2h = apool.tile([128, B], BF16, name="d2h0", tag="d2h0")
            nc.vector.tensor_scalar_mul(d2h[0:w0, :], r0[0:w0, :],
                                        w0cn[0:w0, :])

            # h/dh/d2h state: lists of (tile, chunk list) in layer-major form
            # stored as single tiles with chunks side by side in free dim.
            prev_h, prev_dh, prev_d2h = h, dh, d2h
            prev_chunks = _chunks(w0)

            # ---------------- layers 1..4 (tanh) -------------------------
            for l in range(1, 5):
                fi, fo = LAYERS[l], LAYERS[l + 1]
                kcs = _chunks(fi)
                mcs = _chunks(fo)
                nmc = len(mcs)
                h_n = apool.tile([128, nmc * B], F32R, name=f"h{l}",
                                 tag=f"h{l}")
                dh_n = apool.tile([128, nmc * B], BF16, name=f"dh{l}",
                                  tag=f"dh{l}")
                d2h_n = apool.tile([128, nmc * B], BF16, name=f"d2h{l}",
                                   tag=f"d2h{l}")
                for mi, (mo, ms) in enumerate(mcs):
                    ph = pmm.tile([128, B], F32, name=f"ph{l}_{mi}", tag="ph")
                    pdh = pmm.tile([128, B], F32, name=f"pdh{l}_{mi}",
                                   tag="pdh")
                    pd2h = pmm.tile([128, B], F32, name=f"pd2h{l}_{mi}",
                                    tag="pd2h")
                    for ki, (ko, ks) in enumerate(kcs):
                        st, sp = ki == 0, ki == len(kcs) - 1
                        wsl = slice(ki * fo + mo, ki * fo + mo + ms)
                        rsl = slice(ki * B, ki * B + B)
                        nc.tensor.matmul(ph[0:ms, :], wtr[l][0:ks, wsl],
                                         prev_h[0:ks, rsl], start=st, stop=sp)
                        nc.tensor.matmul(pdh[0:ms, :], wtb[l][0:ks, wsl],
                                         prev_dh[0:ks, rsl], start=st, stop=sp)
                        nc.tensor.matmul(pd2h[0:ms, :], wtb[l][0:ks, wsl],
                                         prev_d2h[0:ks, rsl], start=st,
                                         stop=sp)
                    osl = slice(mi * B, mi * B + B)
                    # value stream
                    nc.scalar.activation(h_n[0:ms, osl], ph[0:ms, :], AF.Tanh,
                                         bias=bc[l][0:ms, mi:mi + 1])
                    tbf = tpool.tile([128, B], BF16, name=f"tbf{l}_{mi}",
                                     tag="tbf")
                    nc.gpsimd.tensor_copy(tbf[0:ms, :],
                                          h_n[0:ms, osl].bitcast(F32))
                    ttb = tpool.tile([128, B], BF16, name=f"ttb{l}_{mi}",
                                     tag="ttb")
                    nc.scalar.activation(ttb[0:ms, :],
                                         h_n[0:ms, osl].bitcast(F32),
                                         AF.Square)
                    sbf = tpool.tile([128, B], BF16, name=f"sbf{l}_{mi}",
                                     tag="sbf")
                    nc.gpsimd.tensor_scalar(sbf[0:ms, :], ttb[0:ms, :], -1.0,
                                            1.0, ALU.mult, ALU.add)
                    dab = tpool.tile([128, B], BF16, name=f"dab{l}_{mi}",
                                     tag="dab")
                    nc.scalar.activation(dab[0:ms, :], pdh[0:ms, :], AF.Copy)
                    nc.vector.tensor_mul(dh_n[0:ms, osl], sbf[0:ms, :],
                                         dab[0:ms, :])
                    pbf = tpool.tile([128, B], BF16, name=f"pbf{l}_{mi}",
                                     tag="pbf")
                    nc.vector.tensor_mul(pbf[0:ms, :], dh_n[0:ms, osl],
                                         dab[0:ms, :])
                    rbf = tpool.tile([128, B], BF16, name=f"rbf{l}_{mi}",
                                     tag="rbf")
                    nc.vector.tensor_mul(rbf[0:ms, :], tbf[0:ms, :],
                                         pbf[0:ms, :])
                    qbf = tpool.tile([128, B], BF16, name=f"qbf{l}_{mi}",
                                     tag="qbf")
                    nc.vector.tensor_mul(qbf[0:ms, :], sbf[0:ms, :],
                                         pd2h[0:ms, :])
                    nc.vector.scalar_tensor_tensor(d2h_n[0:ms, osl],
                                                   rbf[0:ms, :], -2.0,
                                                   qbf[0:ms, :], ALU.mult,
                                                   ALU.add)
                prev_h, prev_dh, prev_d2h = h_n, dh_n, d2h_n
                prev_chunks = mcs

            # ------------- layer 5 (200 -> 100, batch-major) -------------
            fi = LAYERS[5]
            kcs = _chunks(fi)  # [(0,128),(128,72)]
            ffeat = tpool.tile([128, B], F32R, name="ffeat", tag="ffeat")
            u_sb = tpool.tile([128, 4 * Q], F32, name="u_sb", tag="u_sb")
            for m in range(4):  # batch sub-chunks of 128
                msl = slice(m * 128, (m + 1) * 128)
                po = pmm.tile([128, Q], F32, name=f"po{m}", tag="ph")
                pdo = pmm.tile([128, Q], F32, name=f"pdo{m}", tag="pdh")
                pd2o = pmm.tile([128, Q], F32, name=f"pd2o{m}", tag="pd2h")
                for ki, (ko, ks) in enumerate(kcs):
                    st, sp = ki == 0, ki == len(kcs) - 1
                    wsl = slice(ki * Q, ki * Q + Q)
                    nc.tensor.matmul(po[:, :], prev_h[0:ks, ki * B + m * 128:
                                                      ki * B + (m + 1) * 128],
                                     wtr[5][0:ks, wsl], start=st, stop=sp)
                    nc.tensor.matmul(pdo[:, :], prev_dh[0:ks, ki * B + m * 128:
                                                        ki * B + (m + 1) * 128],
                                     wtb[5][0:ks, wsl], start=st, stop=sp)
                    nc.tensor.matmul(pd2o[:, :],
                                     prev_d2h[0:ks, ki * B + m * 128:
                                              ki * B + (m + 1) * 128],
                                     wtb[5][0:ks, wsl], start=st, stop=sp)
                xi = t * 4 + m  # column index into xcol/xsq1/x4
                usl = slice(m * Q, (m + 1) * Q)
                # u = (x^2-1)*out - 1
                nc.scalar.activation(u_sb[:, usl], po[:, :], AF.Copy,
                                     bias=-1.0, scale=xsq1[:, xi:xi + 1])
                u2 = tpool.tile([128, Q], F32, name=f"u2_{m}", tag="u2")
                nc.scalar.activation(u2[:, :], u_sb[:, usl], AF.Square)
                g = tpool.tile([128, Q], F32, name=f"g_{m}", tag="g")
                nc.vector.scalar_tensor_tensor(g[:, :], u2[:, :], -1.0,
                                               u_sb[:, usl], ALU.add,
                                               ALU.mult)
                ds1 = tpool.tile([128, Q], BF16, name=f"ds1_{m}", tag="ds1")
                nc.scalar.activation(ds1[:, :], pdo[:, :], AF.Copy,
                                     scale=x4[:, xi:xi + 1])
                ds2 = tpool.tile([128, Q], BF16, name=f"ds2_{m}", tag="ds2")
                nc.scalar.activation(ds2[:, :], pd2o[:, :], AF.Copy,
                                     scale=xsq1[:, xi:xi + 1])
                q1 = tpool.tile([128, Q], BF16, name=f"q1_{m}", tag="q1")
                nc.vector.scalar_tensor_tensor(q1[:, :], po[:, :], 2.0,
                                               ds1[:, :], ALU.mult, ALU.add)
                d2u = tpool.tile([128, Q], BF16, name=f"d2u_{m}", tag="d2u")
                nc.vector.tensor_add(d2u[:, :], q1[:, :], ds2[:, :])
                h1 = tpool.tile([128, Q], F32, name=f"h1_{m}", tag="h1")
                nc.vector.scalar_tensor_tensor(h1[:, :], d2u[:, :], -1e-4,
                                               g[:, :], ALU.mult, ALU.add)
                # transpose h1 (128b x 100q) -> (100q x 128b) feature-major
                ptr = pmisc.tile([128, 128], F32, name=f"ptr{m}", tag="ptr")
                nc.tensor.transpose(ptr[0:Q, :], h1[:, :], ident[:, :])
                nc.scalar.activation(ffeat[0:Q, msl], ptr[0:Q, :], AF.Copy)
            # IRK matmuls + final add, batch-major out
            for m in range(4):
                msl = slice(m * 128, (m + 1) * 128)
                usl = slice(m * Q, (m + 1) * Q)
                pug = pmisc.tile([128, 2 * Q], F32, name=f"pug{m}", tag="pug")
                nc.tensor.matmul(pug[:, :], ffeat[0:Q, msl], g12[0:Q, :],
                                 start=True, stop=True)
                ou = tpool.tile([128, 2 * Q], F32, name=f"ou{m}", tag="ou")
                nc.vector.tensor_add(ou[:, 0:Q], pug[:, 0:Q], u_sb[:, usl])
                nc.vector.tensor_add(ou[:, Q:2 * Q], pug[:, Q:2 * Q],
                                     u_sb[:, usl])
                n0 = t * B + m * 128
                nc.gpsimd.dma_start(out=u0_e[n0:n0 + 128, :],
                                    in_=ou[:, 0:Q])
                nc.gpsimd.dma_start(out=u1_e[n0:n0 + 128, :],
                                    in_=ou[:, Q:2 * Q])

    nc.compile()
    return nc


def prep_inputs(W, b, x, A, bvec):
    """Host-side weight/layout prep (weight-sized work only). Returns the
    common (replicated) input map and the per-core x shards."""
    common = {}
    for l in range(1, 6):
        fi, fo = LAYERS[l], LAYERS[l + 1]
        kcs = _chunks(fi)
        wt = np.zeros((128, len(kcs) * fo), np.float32)
        for ki, (ko, ks) in enumerate(kcs):
            wt[0:ks, ki * fo:(ki + 1) * fo] = W[l].T[ko:ko + ks, :]
        common[f"wtr{l}"] = wt
        common[f"wtb{l}"] = wt.astype(ml_dtypes.bfloat16)
        mcs = _chunks(fo)
        bcol = np.zeros((128, len(mcs)), np.float32)
        for mi, (mo, ms) in enumerate(mcs):
            bcol[0:ms, mi] = b[l][mo:mo + ms]
        common[f"bc{l}"] = bcol
    common["w0r"] = W[0].T.astype(np.float32)  # (1, 20)
    w0col = np.zeros((128, 1), np.float32)
    w0col[0:20, 0] = W[0][:, 0]
    common["w0cb"] = w0col
    common["w0cn"] = (-2.0 * w0col).astype(np.float32)
    b0col = np.zeros((128, 1), np.float32)
    b0col[0:20, 0] = b[0]
    common["b0c"] = b0col
    g12 = np.zeros((128, 2 * Q), np.float32)
    ahat = (5.0 * DT) * A.T                      # (q, j) = 5*DT*A[j, q]
    a2hat = (5.0 * DT) * (A - np.ones((Q, 1)) @ bvec).T
    g12[0:Q, 0:Q] = ahat
    g12[0:Q, Q:2 * Q] = a2hat
    common["g12"] = g12

    xs = x.reshape(N_CORES, NC)
    shards = []
    for c in range(N_CORES):
        m = {"xrow": xs[c].reshape(T, B),
             "xcol": xs[c].reshape(XC, 128).T.copy()}
        shards.append(m)
    return common, shards


_NC_CACHE = None


def kernel(W0, b0, W1, b1, W2, b2, W3, b3, W4, b4, W5, b5, x, A, bvec):
    global _NC_CACHE
    W = [np.asarray(w, np.float32) for w in (W0, W1, W2, W3, W4, W5)]
    bs = [np.asarray(v, np.float32) for v in (b0, b1, b2, b3, b4, b5)]
    x = np.asarray(x, np.float32)
    A = np.asarray(A, np.float32)
    bvec = np.asarray(bvec, np.float32)

    if _NC_CACHE is None:
        _NC_CACHE = build_kernel()
    nc = _NC_CACHE

    common, shards = prep_inputs(W, bs, x, A, bvec)
    in_maps = [{**common, **shards[c]} for c in range(N_CORES)]

    from concourse.bass_utils import run_bass_kernel_spmd
    res = run_bass_kernel_spmd(nc, in_maps, list(range(N_CORES)))
    U0 = np.concatenate([res.results[c]["U0"] for c in range(N_CORES)], 0)
    U1 = np.concatenate([res.results[c]["U1"] for c in range(N_CORES)], 0)
    return U0, U1
